# revision 45
# baseline (speedup 1.0000x reference)
"""Memory-Network kernel for 8 Trainium2 NeuronCores.

Data-parallel: batch B=128 is split 16-per-core; each core processes its
160 (b, r) sequences end-to-end (embedding gather, q/f LSTMs, attention,
FC) with no collectives. Weights are replicated; all layout prep
(transposes, gate permutation, bf16 casts, padding) happens on host.

Device layout convention: everything feature-major [feature, token] so
the LSTM recurrence's matmuls keep gates/hidden on the partition dim.

Per-core embedding rows are host-deduplicated into one compact table
(<= 9728 rows, int16-indexable) so each token needs a single gather.
The two LSTMs are interleaved 1q:2f so both finish together and each
stream's elementwise chain hides under the other's matmuls.

All matmuls are bf16: the recurrence is weight-load-rate-bound and the
fp8 DoubleRow/SwInterleave modes serialize LDWEIGHTS with the stream on
this hardware (measured 2.6x worse per gate-tile). The x-side third
K-chunk runs as an exact 64-row tile (features 256..299 + bias) to cut
weight-load rows; K must stay in {32, 64, 128} - partial tiles leave
PE rows undriven and corrupt results.
"""

import sys

for _p in ("/opt/trn_rl_repo", "/root/.axon_site/_ro/trn_rl_repo"):
    if _p not in sys.path:
        sys.path.insert(0, _p)

import numpy as np
import ml_dtypes

import concourse.bass as bass
import concourse.mybir as mybir
import concourse.tile as tile
from concourse import bacc
from concourse.bass_utils import run_bass_kernel_spmd
from concourse.masks import make_identity

BF16 = mybir.dt.bfloat16
F32 = mybir.dt.float32
I16 = mybir.dt.int16

NP_BF16 = ml_dtypes.bfloat16

VOCAB, EMB, HID, IMG = 50000, 300, 512, 4096
B, R, LQ, LH = 128, 10, 20, 40
N_CORES = 8
BS = B // N_CORES          # 16 batch items per core
S = BS * R                 # 160 sequences per core
EPAD = 384                 # embedding row padded to 3x128 for K-chunking
KX2 = 128                  # x-chunk-2 rows kept at the full 128 tile:
                           # K<128 tiles trigger a PE tile-size reconfig
                           # between instructions (~45us/kernel penalty)
G4 = 4 * HID               # 2048 gate rows
NEG = -1.0e30
NU_PAD = 9728              # compact per-core embedding table rows (>= uniques)

_STATE = None


def _gate_perm():
    # m-tile m = 4*j + pos: hidden chunk j's gates in order (i, f, o, g) so
    # the three sigmoids sit in adjacent PSUM banks (one batched ACT op)
    return np.concatenate(
        [np.arange(g * HID + j * 128, g * HID + (j + 1) * 128)
         for j in range(4) for g in (0, 1, 3, 2)]
    )


def _build_program():
    nc = bacc.Bacc(num_swdge_queues=4)

    dt_in = {}

    def din(name, shape, dtype):
        dt_in[name] = nc.dram_tensor(name, list(shape), dtype, kind="ExternalInput")
        return dt_in[name]

    embC_d = din("embC", [NU_PAD, EPAD], BF16)
    idxq_d = din("idxq", [128, LQ * S // 16], I16)   # [128, 200]
    idxf_d = din("idxf", [128, LH * S // 16], I16)   # [128, 400]
    # first embedding groups (q0, f0, f1) are host-pregathered and DMAed
    # directly: the gpsimd gather path has ~20us of first-use latency
    e0q_d = din("e0q", [128, 3 * 4 * S], BF16)
    e0f_d = din("e0f", [128, 3 * 4 * S], BF16)
    e1f_d = din("e1f", [128, 3 * 4 * S], BF16)
    wqx_d = din("wqx", [128, 3, G4], BF16)
    wqh_d = din("wqh", [128, 4, G4], BF16)
    wfx_d = din("wfx", [128, 3, G4], BF16)
    wfh_d = din("wfh", [128, 4, G4], BF16)
    w1i_d = din("w1i", [128, IMG // 128, HID], BF16)
    w1h_d = din("w1h", [128, 4, HID], BF16)
    b1_d = din("b1", [128, 4], F32)
    w2_d = din("w2", [128, 4, HID], BF16)
    b2_d = din("b2", [128, 4], F32)
    img_d = din("imgrep", [128, IMG // 128, S], BF16)
    mask_d = din("mask", [S, S], F32)
    out_d = nc.dram_tensor("out", [HID, S], F32, kind="ExternalOutput")

    with tile.TileContext(nc) as tc:
        with (
            tc.tile_pool(name="consts", bufs=1) as cp,
            tc.tile_pool(name="hstate", bufs=12) as hp,
            tc.tile_pool(name="cstate", bufs=8) as cpool,
            tc.tile_pool(name="ew", bufs=16) as ew,
            tc.tile_pool(name="w1s", bufs=2) as w1p,
            tc.tile_pool(name="outp", bufs=3) as op,
            tc.tile_pool(name="ps", bufs=2, space="PSUM") as ps,
        ):
            # ---------- phase 0: index/weight DMAs, gathers ----------
            # sync queue: q-stream + late-use consts; scalar queue: f-stream.
            # First-needed data first: x-weights (split by gate half) and the
            # host-pregathered window-0 embedding groups (the gpsimd gather
            # path has ~20us of first-use latency).
            GRP = 4 * S
            wq_sb = cp.tile([128, 7, G4], BF16, name="wq_sb", tag="wq")
            wf_sb = cp.tile([128, 7, G4], BF16, name="wf_sb", tag="wf")
            e0q = cp.tile([128, 3, GRP], BF16, name="e0q", tag="e0q")
            e0f = cp.tile([128, 3, GRP], BF16, name="e0f", tag="e0f")
            e1f = cp.tile([128, 3, GRP], BF16, name="e1f", tag="e1f")
            HG = G4 // 2
            nc.sync.dma_start(
                wq_sb[:, 0:3, 0:HG],
                wqx_d.ap()[:, :, 0:HG])
            nc.sync.dma_start(e0q[:], e0q_d.ap()[:])
            nc.sync.dma_start(
                wq_sb[:, 0:3, HG:G4],
                wqx_d.ap()[:, :, HG:G4])
            nc.sync.dma_start(
                wq_sb[:, 3:7, :],
                wqh_d.ap()[:])
            nc.scalar.dma_start(
                wf_sb[:, 0:3, 0:HG],
                wfx_d.ap()[:, :, 0:HG])
            nc.scalar.dma_start(e0f[:], e0f_d.ap()[:])
            nc.scalar.dma_start(
                wf_sb[:, 0:3, HG:G4],
                wfx_d.ap()[:, :, HG:G4])
            nc.scalar.dma_start(e1f[:], e1f_d.ap()[:])
            nc.scalar.dma_start(
                wf_sb[:, 3:7, :],
                wfh_d.ap()[:])

            idxq_sb = cp.tile(list(idxq_d.shape), I16, name="idxq_sb", tag="idxq")
            nc.sync.dma_start(idxq_sb[:], idxq_d.ap()[:])
            idxf_sb = cp.tile(list(idxf_d.shape), I16, name="idxf_sb", tag="idxf")
            nc.scalar.dma_start(idxf_sb[:], idxf_d.ap()[:])

            # late-use consts; sync engine runs ahead, transfers overlap LSTM
            w1h_sb = cp.tile([128, 4, HID], BF16, name="w1h_sb", tag="w1h")
            nc.sync.dma_start(
                w1h_sb[:], w1h_d.ap()[:])
            w2_sb = cp.tile([128, 4, HID], BF16, name="w2_sb", tag="w2")
            nc.sync.dma_start(
                w2_sb[:], w2_d.ap()[:])
            b1_sb = cp.tile([128, 4], F32, name="b1_sb", tag="b1")
            nc.sync.dma_start(b1_sb[:], b1_d.ap()[:])
            b2_sb = cp.tile([128, 4], F32, name="b2_sb", tag="b2")
            nc.sync.dma_start(b2_sb[:], b2_d.ap()[:])
            img_sb = cp.tile([128, IMG // 128, S], BF16, name="img_sb", tag="img")
            nc.sync.dma_start(
                img_sb[:], img_d.ap()[:])
            mask_sb = cp.tile([128, 2, S], F32, name="mask_sb", tag="mask")
            nc.sync.dma_start(
                mask_sb[:, 0, :], mask_d.ap()[0:128, :])
            nc.sync.dma_start(
                mask_sb[0:S - 128, 1, :], mask_d.ap()[128:S, :])

            ident = cp.tile([128, 128], BF16, name="ident", tag="ident")
            make_identity(nc, ident[:])

            # gathered embeddings, feature-major [128, 3 chunks, cols], one
            # tile per 4-step group (GRP cols). dma_gather(transpose=True)
            # writes feature-major directly from the compact table. Groups
            # q0/f0/f1 came in as direct DMAs above.
            NIC = GRP // 16               # idx columns per group
            eq = [None] * (LQ // 4)
            ef = [None] * (LH // 4)
            eq[0] = e0q
            ef[0] = e0f
            ef[1] = e1f

            qn_ctr = [0]

            def gather_group(idx_sb, g, dst, tag):
                qn = qn_ctr[0] % 4
                qn_ctr[0] += 1
                t_ = cp.tile([128, 3, GRP], BF16, name=f"{tag}{g}",
                             tag=f"{tag}{g}")
                dst[g] = t_
                nc.gpsimd.dma_gather(
                    out_ap=t_[:],
                    in_ap=embC_d.ap()[:],
                    idxs_ap=idx_sb[:, g * NIC:(g + 1) * NIC],
                    num_idxs=GRP,
                    num_idxs_reg=GRP,
                    elem_size=EPAD,
                    transpose=True,
                    queue_num=qn,
                )

            # feed order matches 1q:2f consumption (window w uses eq[w//4],
            # ef[w//2]); first-use deadlines: eq[i] at window 4i, ef[j] at 2j
            feed = sorted(
                [("q", i, 4 * i) for i in range(1, LQ // 4)] +
                [("f", j, 2 * j) for j in range(2, LH // 4)],
                key=lambda x: x[2])
            for (st, g, _) in feed:
                if st == "q":
                    gather_group(idxq_sb, g, eq, "eqg")
                else:
                    gather_group(idxf_sb, g, ef, "efg")

            # ---------- LSTM recurrence ----------
            # One step of one stream. Gate biases are folded into the x-side
            # matmul (embedding col 300 is 1.0; weight row 300 = bias; x
            # chunk 2 is an exact K=64 tile, rows 256..319).
            # Gates per hidden chunk j land in one 4-bank PSUM tile in order
            # (i, f, o, g) so the three sigmoids are one strided ACT op.
            def lstm_step(state, t, e_chunks, w_sb, label):
                h, c_st = state
                ec = e_chunks[t // 4]
                co = (t % 4) * S
                rhs_list = [ec[:, 0, co:co + S], ec[:, 1, co:co + S],
                            ec[0:KX2, 2, co:co + S]]
                nk = 3
                if t > 0:
                    rhs_list += [h[:, j, :] for j in range(4)]
                    nk = 7
                new_h = hp.tile([128, 4, S], BF16, name="hn", tag="h", bufs=4)
                new_c = cpool.tile([128, 4, S], F32, name="cn", tag="c", bufs=4)

                def mm(pg, j, ki):
                    for g in range(4):
                        m = 4 * j + g
                        nc.tensor.matmul(
                            pg[:, g, :],
                            lhsT=w_sb[0:KX2 if ki == 2 else 128, ki,
                                      m * 128:(m + 1) * 128],
                            rhs=rhs_list[ki],
                            start=(ki == 0),
                            stop=(ki == nk - 1),
                        )

                def elementwise(pg, j):
                    sig = ew.tile([128, 3, S], F32, name="sig", tag="sig", bufs=6)
                    nc.scalar.activation(
                        sig[:], pg[:, 0:3, :],
                        mybir.ActivationFunctionType.Sigmoid)
                    tg = ew.tile([128, S], F32, name="tg", tag="ew")
                    nc.scalar.activation(
                        tg[:], pg[:, 3, :], mybir.ActivationFunctionType.Tanh)
                    cn = new_c[:, j, :]
                    if t == 0:
                        nc.vector.tensor_mul(cn, sig[:, 0, :], tg[:])
                    else:
                        m1 = ew.tile([128, S], F32, name="m1", tag="ew")
                        nc.vector.tensor_mul(m1[:], sig[:, 1, :], c_st[:, j, :])
                        m2 = ew.tile([128, S], F32, name="m2", tag="ew")
                        nc.vector.tensor_mul(m2[:], sig[:, 0, :], tg[:])
                        nc.vector.tensor_add(cn, m1[:], m2[:])
                    tc_ = ew.tile([128, S], F32, name="tc", tag="ew")
                    nc.scalar.activation(
                        tc_[:], cn, mybir.ActivationFunctionType.Tanh)
                    nc.vector.tensor_mul(new_h[:, j, :], sig[:, 2, :], tc_[:])

                for j in range(4):
                    pg = ps.tile([128, 4, S], F32, name=f"pg{label}",
                                 tag="pg", padded_shape=[128, 4, 512])
                    for ki in range(nk):
                        mm(pg, j, ki)
                    elementwise(pg, j)
                return (new_h, new_c)

            # interleave 1q:2f so the two streams hide each other's
            # elementwise chains and finish together (no solo tail)
            st_q = (None, None)
            st_f = (None, None)
            for w in range(LQ):
                st_q = lstm_step(st_q, w, eq, wq_sb, "q")
                st_f = lstm_step(st_f, 2 * w, ef, wf_sb, "f")
                st_f = lstm_step(st_f, 2 * w + 1, ef, wf_sb, "f")
            hq_t = st_q[0]
            hq = [hq_t[:, j, :] for j in range(4)]
            hf_t = st_f[0]
            hf = [hf_t[:, j, :] for j in range(4)]

            # ---------- query = tanh([img, hq] @ W1.T + b1) ----------
            pq = ps.tile([128, 4, S], F32, name="pq", tag="pg",
                         padded_shape=[128, 4, 512])

            def qslice(m):
                return pq[:, m, :]

            # 16 streamed lhsT blocks of 2 k-chunks, alternating DMA queues
            # (a single in-order queue starves the matmuls at 2.9us/block)
            n_im_blk = IMG // 256
            for bI in range(n_im_blk):
                w1c = w1p.tile([128, 2, HID], BF16, name="w1c", tag="w1c",
                               bufs=4)
                eng = nc.sync if bI % 2 == 0 else nc.scalar
                eng.dma_start(
                    w1c[:], w1i_d.ap()[:, 2 * bI:2 * bI + 2, :])
                for k8 in range(2):
                    ki = bI * 2 + k8
                    for m in range(4):
                        nc.tensor.matmul(
                            qslice(m),
                            lhsT=w1c[:, k8, m * 128:(m + 1) * 128],
                            rhs=img_sb[:, ki, :],
                            start=(ki == 0),
                            stop=False,
                        )
            for k in range(4):
                for m in range(4):
                    nc.tensor.matmul(
                        qslice(m),
                        lhsT=w1h_sb[:, k, m * 128:(m + 1) * 128],
                        rhs=hq[k][:],
                        start=False,
                        stop=(k == 3),
                    )
            qt_f = []
            qt_b = []
            for m in range(4):
                qf = cp.tile([128, S], F32, name=f"qtf{m}", tag=f"qtf{m}")
                nc.scalar.activation(
                    qf[:], qslice(m), mybir.ActivationFunctionType.Tanh,
                    bias=b1_sb[:, m:m + 1])
                qb = cp.tile([128, S], BF16, name=f"qtb{m}", tag=f"qtb{m}")
                nc.vector.tensor_copy(qb[:], qf[:])
                qt_f.append(qf)
                qt_b.append(qb)

            # ---------- attention ----------
            # scores[n, n'] = sum_h Q[h, n] hf[h, n']  (2 partition tiles of n)
            sct = ps.tile([128, 4, S], F32, name="sct", tag="pg",
                          padded_shape=[128, 4, 512])
            sc0, sc1 = sct[:, 0, :], sct[0:S - 128, 1, :]
            for k in range(4):
                nc.tensor.matmul(sc0, lhsT=qt_b[k][:, 0:128], rhs=hf[k][:],
                                 start=(k == 0), stop=(k == 3))
            for k in range(4):
                nc.tensor.matmul(sc1, lhsT=qt_b[k][:, 128:S], rhs=hf[k][:],
                                 start=(k == 0), stop=(k == 3))

            # scores are tiny (|s| < 1: h states are ~0.03-scale), so the
            # softmax max-subtraction is skipped; exp(-1e30 mask) -> 0
            a_bf = []  # attention weights, 2 partition tiles [*, S] bf16
            for ti, (scp, npart) in enumerate([(sc0, 128), (sc1, S - 128)]):
                sm = ew.tile([128, S], F32, name="sm", tag="ew")
                nc.vector.tensor_add(sm[:npart], scp, mask_sb[:npart, ti, :])
                ex = ew.tile([128, S], F32, name="ex", tag="ew")
                nc.scalar.activation(
                    ex[:npart], sm[:npart], mybir.ActivationFunctionType.Exp)
                ssum = ew.tile([128, 1], F32, name="ssum", tag="red", bufs=4)
                nc.vector.tensor_reduce(
                    ssum[:npart], ex[:npart], mybir.AxisListType.X,
                    mybir.AluOpType.add)
                rs = ew.tile([128, 1], F32, name="rs", tag="red", bufs=4)
                nc.vector.reciprocal(rs[:npart], ssum[:npart])
                ab = ew.tile([128, S], BF16, name="ab", tag="abf", bufs=8)
                nc.vector.tensor_scalar_mul(ab[:npart], ex[:npart], rs[:npart])
                a_bf.append(ab)

            # A^T (s'-major) via PE transpose; 2 tiles covering s' 0:128, 128:160
            at = [cp.tile([128, S], BF16, name=f"at{i}", tag=f"at{i}")
                  for i in range(2)]
            blocks = [  # (src tile idx, src col slice, dst tile idx, dst col off)
                (0, 0, 128, 0, 0),
                (1, 0, 128, 0, 128),
                (0, 128, S, 1, 0),
                (1, 128, S, 1, 128),
            ]
            for (sti, c0, c1, dti, dc) in blocks:
                src = a_bf[sti]
                np_src = 128 if sti == 0 else S - 128
                w_ = c1 - c0
                pt = ps.tile([128, S], BF16, name="pt", tag="pg")
                nc.tensor.transpose(
                    pt[0:w_, 0:np_src], src[0:np_src, c0:c1],
                    ident[0:np_src, 0:np_src])
                nc.vector.tensor_copy(
                    at[dti][0:w_, dc:dc + np_src], pt[0:w_, 0:np_src])

            # hf token-major [S, 512] as 2 partition tiles
            hft = [cp.tile([128, 4, 128], BF16, name=f"hft{i}", tag=f"hft{i}")
                   for i in range(2)]
            for k in range(4):
                pt = ps.tile([128, S], BF16, name="pt2", tag="pg")
                nc.tensor.transpose(
                    pt[0:128, 0:128], hf[k][:, 0:128], ident[:])
                nc.vector.tensor_copy(hft[0][:, k, :], pt[0:128, 0:128])
                pt = ps.tile([128, S], BF16, name="pt3", tag="pg")
                nc.tensor.transpose(
                    pt[0:S - 128, 0:128], hf[k][:, 128:S], ident[:])
                nc.vector.tensor_copy(
                    hft[1][0:S - 128, k, :], pt[0:S - 128, 0:128])

            # att_hist^T [512, S] = hf^T(feature-major result) : contract over s'
            att_b = []
            pa = ps.tile([128, 4, S], F32, name="pa", tag="pg",
                         padded_shape=[128, 4, 512])
            for m in range(4):
                nc.tensor.matmul(pa[:, m, :], lhsT=hft[0][:, m, :], rhs=at[0][:],
                                 start=True, stop=False)
                nc.tensor.matmul(pa[:, m, :], lhsT=hft[1][0:S - 128, m, :],
                                 rhs=at[1][0:S - 128, :],
                                 start=False, stop=True)
                ab2 = ew.tile([128, S], BF16, name="ab2", tag="abf", bufs=8)
                nc.vector.tensor_copy(ab2[:], pa[:, m, :])
                att_b.append(ab2)

            # out = Q + tanh(att @ W2.T + b2), feature-major [512, S]
            po = ps.tile([128, 4, S], F32, name="po", tag="pg",
                         padded_shape=[128, 4, 512])
            for m in range(4):
                for k in range(4):
                    nc.tensor.matmul(
                        po[:, m, :],
                        lhsT=w2_sb[:, k, m * 128:(m + 1) * 128],
                        rhs=att_b[k][:],
                        start=(k == 0), stop=(k == 3))
                th = ew.tile([128, S], F32, name="th", tag="ew")
                nc.scalar.activation(
                    th[:], po[:, m, :], mybir.ActivationFunctionType.Tanh,
                    bias=b2_sb[:, m:m + 1])
                om = op.tile([128, S], F32, name="om", tag="om")
                nc.vector.tensor_add(om[:], th[:], qt_f[m][:])
                nc.sync.dma_start(out_d.ap()[m * 128:(m + 1) * 128, :], om[:])

    nc.compile()
    return nc


def _prep_shared(inp):
    f32 = np.float32
    emb = np.asarray(inp["emb"], f32)
    embp = np.zeros((VOCAB, EPAD), NP_BF16)
    embp[:, :EMB] = emb.astype(NP_BF16)
    embp[0, :] = 0
    embp[:, EMB] = 1  # ones column (feature 300): x-matmul adds the bias row

    perm = _gate_perm()

    def pmaj(a):
        # [K, M] -> partition-major [128, K//128, M] so DMAs are contiguous
        k, m_ = a.shape
        return np.ascontiguousarray(
            a.reshape(k // 128, 128, m_).transpose(1, 0, 2))

    def fuse_w(wih, whh, bih, bhh):
        wx = np.zeros((EPAD, G4), f32)
        wx[0:EMB, :] = np.asarray(wih, f32).T
        wx[EMB, :] = np.asarray(bih, f32) + np.asarray(bhh, f32)
        wh = np.ascontiguousarray(np.asarray(whh, f32).T)
        return (pmaj(wx[:, perm].astype(NP_BF16)),
                pmaj(wh[:, perm].astype(NP_BF16)))

    wqx, wqh = fuse_w(inp["Wih_q"], inp["Whh_q"], inp["bih_q"], inp["bhh_q"])
    wfx, wfh = fuse_w(inp["Wih_f"], inp["Whh_f"], inp["bih_f"], inp["bhh_f"])
    W1 = np.asarray(inp["W1"], f32)
    shared = {
        "wqx": wqx, "wqh": wqh, "wfx": wfx, "wfh": wfh,
        "w1i": pmaj(np.ascontiguousarray(W1[:, :IMG].T).astype(NP_BF16)),
        "w1h": pmaj(np.ascontiguousarray(W1[:, IMG:].T).astype(NP_BF16)),
        "b1": np.ascontiguousarray(
            np.asarray(inp["b1"], f32).reshape(4, 128).T),
        "w2": pmaj(
            np.ascontiguousarray(np.asarray(inp["W2"], f32).T).astype(NP_BF16)),
        "b2": np.ascontiguousarray(
            np.asarray(inp["b2"], f32).reshape(4, 128).T),
        "_embp": embp,
    }
    n = np.arange(S)
    mask = np.where(
        (n[:, None] // R == n[None, :] // R) & (n[None, :] % R <= n[:, None] % R),
        np.float32(0.0), np.float32(NEG))
    shared["mask"] = np.ascontiguousarray(mask.astype(f32))
    return shared


def _prep_core(inp, core, embp):
    sl = slice(core * BS, (core + 1) * BS)

    def flat(arr, L):
        # t-major flat order i = t*S + n; dma_gather reads index i from
        # [i % 16, base + i // 16], 16-partition block replicated to 128
        return np.asarray(arr[sl], np.int64).reshape(S, L).T.reshape(-1)

    qf = flat(inp["questions"], LQ)          # [3200]
    ff = flat(inp["history"], LH)            # [6400]
    uniq, inv = np.unique(np.concatenate([qf, ff]), return_inverse=True)
    assert len(uniq) <= NU_PAD
    embC = np.zeros((NU_PAD, EPAD), NP_BF16)
    embC[:len(uniq)] = embp[uniq]
    inv = inv.astype(np.int16)

    def wrap(x):
        w = x.reshape(-1, 16).T                       # [16, L*S/16]
        return np.ascontiguousarray(np.tile(w, (8, 1)))

    def pregather(flat_idx):
        # same layout dma_gather(transpose=True) writes: [128, 3, GRP] with
        # out[p, c, i] = row_i[c*128 + p]
        rows = embC[flat_idx]                      # [GRP, 384]
        return np.ascontiguousarray(
            rows.reshape(-1, 3, 128).transpose(2, 1, 0).reshape(128, -1))

    GRP = 4 * S
    qi = inv[:LQ * S]
    fi = inv[LQ * S:]
    img = np.asarray(inp["img_features"], np.float32)[sl]          # [16, 4096]
    img_rep = np.repeat(img, R, axis=0).T                          # [4096, 160]
    return {
        "embC": embC,
        "idxq": wrap(qi),
        "idxf": wrap(fi),
        "e0q": pregather(qi[:GRP]),
        "e0f": pregather(fi[:GRP]),
        "e1f": pregather(fi[GRP:2 * GRP]),
        "imgrep": np.ascontiguousarray(
            img_rep.astype(NP_BF16).reshape(32, 128, S).transpose(1, 0, 2)),
    }


def kernel(**inputs) -> np.ndarray:
    global _STATE
    if _STATE is None:
        _STATE = _build_program()
    nc = _STATE

    shared = _prep_shared(inputs)
    embp = shared.pop("_embp")
    in_maps = []
    for c in range(N_CORES):
        m = dict(shared)
        m.update(_prep_core(inputs, c, embp))
        in_maps.append(m)

    res = run_bass_kernel_spmd(nc, in_maps, core_ids=list(range(N_CORES)))
    outs = []
    for c in range(N_CORES):
        o = np.asarray(res.results[c]["out"], np.float32)   # [512, 160]
        outs.append(o.T.reshape(BS, R, HID))
    return np.concatenate(outs, axis=0)                      # [128, 10, 512]


# revision 46
# speedup vs baseline: 1.0063x; 1.0063x over previous
"""Memory-Network kernel for 8 Trainium2 NeuronCores.

Data-parallel: batch B=128 is split 16-per-core; each core processes its
160 (b, r) sequences end-to-end (embedding gather, q/f LSTMs, attention,
FC) with no collectives. Weights are replicated; all layout prep
(transposes, gate permutation, bf16 casts, padding) happens on host.

Device layout convention: everything feature-major [feature, token] so
the LSTM recurrence's matmuls keep gates/hidden on the partition dim.

Per-core embedding rows are host-deduplicated into one compact table
(<= 9728 rows, int16-indexable) so each token needs a single gather.
The two LSTMs are interleaved 1q:2f so both finish together and each
stream's elementwise chain hides under the other's matmuls.

All matmuls are bf16: the recurrence is weight-load-rate-bound and the
fp8 DoubleRow/SwInterleave modes serialize LDWEIGHTS with the stream on
this hardware (measured 2.6x worse per gate-tile). The x-side third
K-chunk runs as an exact 64-row tile (features 256..299 + bias) to cut
weight-load rows; K must stay in {32, 64, 128} - partial tiles leave
PE rows undriven and corrupt results.
"""

import sys

for _p in ("/opt/trn_rl_repo", "/root/.axon_site/_ro/trn_rl_repo"):
    if _p not in sys.path:
        sys.path.insert(0, _p)

import numpy as np
import ml_dtypes

import concourse.bass as bass
import concourse.mybir as mybir
import concourse.tile as tile
from concourse import bacc
from concourse.bass_utils import run_bass_kernel_spmd
from concourse.masks import make_identity

BF16 = mybir.dt.bfloat16
F32 = mybir.dt.float32
I16 = mybir.dt.int16

NP_BF16 = ml_dtypes.bfloat16

VOCAB, EMB, HID, IMG = 50000, 300, 512, 4096
B, R, LQ, LH = 128, 10, 20, 40
N_CORES = 8
BS = B // N_CORES          # 16 batch items per core
S = BS * R                 # 160 sequences per core
EPAD = 384                 # embedding row padded to 3x128 for K-chunking
KX2 = 128                  # x-chunk-2 rows kept at the full 128 tile:
                           # K<128 tiles trigger a PE tile-size reconfig
                           # between instructions (~45us/kernel penalty)
G4 = 4 * HID               # 2048 gate rows
NEG = -1.0e30
NU_PAD = 9728              # compact per-core embedding table rows (>= uniques)

_STATE = None


def _gate_perm():
    # m-tile m = 4*j + pos: hidden chunk j's gates in order (i, f, o, g) so
    # the three sigmoids sit in adjacent PSUM banks (one batched ACT op)
    return np.concatenate(
        [np.arange(g * HID + j * 128, g * HID + (j + 1) * 128)
         for j in range(4) for g in (0, 1, 3, 2)]
    )


def _build_program():
    nc = bacc.Bacc(num_swdge_queues=4)

    dt_in = {}

    def din(name, shape, dtype):
        dt_in[name] = nc.dram_tensor(name, list(shape), dtype, kind="ExternalInput")
        return dt_in[name]

    embC_d = din("embC", [NU_PAD, EPAD], BF16)
    idxq_d = din("idxq", [128, LQ * S // 16], I16)   # [128, 200]
    idxf_d = din("idxf", [128, LH * S // 16], I16)   # [128, 400]
    # first embedding groups (q0, f0, f1) are host-pregathered and DMAed
    # directly: the gpsimd gather path has ~20us of first-use latency
    e0q_d = din("e0q", [128, 3 * 4 * S], BF16)
    e0f_d = din("e0f", [128, 3 * 4 * S], BF16)
    e1f_d = din("e1f", [128, 3 * 4 * S], BF16)
    wqx_d = din("wqx", [EPAD, G4], BF16)
    wqh_d = din("wqh", [HID, G4], BF16)
    wfx_d = din("wfx", [EPAD, G4], BF16)
    wfh_d = din("wfh", [HID, G4], BF16)
    w1i_d = din("w1i", [IMG, HID], BF16)
    w1h_d = din("w1h", [HID, HID], BF16)
    b1_d = din("b1", [128, 4], F32)
    w2_d = din("w2", [HID, HID], BF16)
    b2_d = din("b2", [128, 4], F32)
    img_d = din("imgrep", [IMG, S], BF16)
    mask_d = din("mask", [S, S], F32)
    out_d = nc.dram_tensor("out", [HID, S], F32, kind="ExternalOutput")

    with tile.TileContext(nc) as tc:
        with (
            tc.tile_pool(name="consts", bufs=1) as cp,
            tc.tile_pool(name="hstate", bufs=12) as hp,
            tc.tile_pool(name="cstate", bufs=8) as cpool,
            tc.tile_pool(name="ew", bufs=16) as ew,
            tc.tile_pool(name="w1s", bufs=2) as w1p,
            tc.tile_pool(name="outp", bufs=3) as op,
            tc.tile_pool(name="ps", bufs=2, space="PSUM") as ps,
        ):
            # ---------- phase 0: index/weight DMAs, gathers ----------
            # sync queue: q-stream + late-use consts; scalar queue: f-stream.
            # First-needed data first: x-weights (split by gate half) and the
            # host-pregathered window-0 embedding groups (the gpsimd gather
            # path has ~20us of first-use latency).
            GRP = 4 * S
            wq_sb = cp.tile([128, 7, G4], BF16, name="wq_sb", tag="wq")
            wf_sb = cp.tile([128, 7, G4], BF16, name="wf_sb", tag="wf")
            e0q = cp.tile([128, 3, GRP], BF16, name="e0q", tag="e0q")
            e0f = cp.tile([128, 3, GRP], BF16, name="e0f", tag="e0f")
            e1f = cp.tile([128, 3, GRP], BF16, name="e1f", tag="e1f")
            HG = G4 // 2
            nc.sync.dma_start(
                wq_sb[:, 0:3, 0:HG],
                wqx_d.ap()[:, 0:HG].rearrange("(k p) m -> p k m", p=128))
            nc.sync.dma_start(e0q[:], e0q_d.ap()[:])
            nc.sync.dma_start(
                wq_sb[:, 0:3, HG:G4],
                wqx_d.ap()[:, HG:G4].rearrange("(k p) m -> p k m", p=128))
            nc.sync.dma_start(
                wq_sb[:, 3:7, :],
                wqh_d.ap().rearrange("(k p) m -> p k m", p=128))
            nc.scalar.dma_start(
                wf_sb[:, 0:3, 0:HG],
                wfx_d.ap()[:, 0:HG].rearrange("(k p) m -> p k m", p=128))
            nc.scalar.dma_start(e0f[:], e0f_d.ap()[:])
            nc.scalar.dma_start(
                wf_sb[:, 0:3, HG:G4],
                wfx_d.ap()[:, HG:G4].rearrange("(k p) m -> p k m", p=128))
            nc.scalar.dma_start(e1f[:], e1f_d.ap()[:])
            nc.scalar.dma_start(
                wf_sb[:, 3:7, :],
                wfh_d.ap().rearrange("(k p) m -> p k m", p=128))

            idxq_sb = cp.tile(list(idxq_d.shape), I16, name="idxq_sb", tag="idxq")
            nc.sync.dma_start(idxq_sb[:], idxq_d.ap()[:])
            idxf_sb = cp.tile(list(idxf_d.shape), I16, name="idxf_sb", tag="idxf")
            nc.scalar.dma_start(idxf_sb[:], idxf_d.ap()[:])

            # late-use consts; sync engine runs ahead, transfers overlap LSTM
            w1h_sb = cp.tile([128, 4, HID], BF16, name="w1h_sb", tag="w1h")
            nc.sync.dma_start(
                w1h_sb[:], w1h_d.ap().rearrange("(k p) m -> p k m", p=128))
            w2_sb = cp.tile([128, 4, HID], BF16, name="w2_sb", tag="w2")
            nc.sync.dma_start(
                w2_sb[:], w2_d.ap().rearrange("(k p) m -> p k m", p=128))
            b1_sb = cp.tile([128, 4], F32, name="b1_sb", tag="b1")
            nc.sync.dma_start(b1_sb[:], b1_d.ap()[:])
            b2_sb = cp.tile([128, 4], F32, name="b2_sb", tag="b2")
            nc.sync.dma_start(b2_sb[:], b2_d.ap()[:])
            img_sb = cp.tile([128, IMG // 128, S], BF16, name="img_sb", tag="img")
            nc.sync.dma_start(
                img_sb[:], img_d.ap().rearrange("(k p) m -> p k m", p=128))
            mask_sb = cp.tile([128, 2, S], F32, name="mask_sb", tag="mask")
            nc.sync.dma_start(
                mask_sb[:, 0, :], mask_d.ap()[0:128, :])
            nc.sync.dma_start(
                mask_sb[0:S - 128, 1, :], mask_d.ap()[128:S, :])

            ident = cp.tile([128, 128], BF16, name="ident", tag="ident")
            make_identity(nc, ident[:])

            # gathered embeddings, feature-major [128, 3 chunks, cols], one
            # tile per 4-step group (GRP cols). dma_gather(transpose=True)
            # writes feature-major directly from the compact table. Groups
            # q0/f0/f1 came in as direct DMAs above.
            NIC = GRP // 16               # idx columns per group
            eq = [None] * (LQ // 4)
            ef = [None] * (LH // 4)
            eq[0] = e0q
            ef[0] = e0f
            ef[1] = e1f

            qn_ctr = [0]

            def gather_group(idx_sb, g, dst, tag):
                qn = qn_ctr[0] % 4
                qn_ctr[0] += 1
                t_ = cp.tile([128, 3, GRP], BF16, name=f"{tag}{g}",
                             tag=f"{tag}{g}")
                dst[g] = t_
                nc.gpsimd.dma_gather(
                    out_ap=t_[:],
                    in_ap=embC_d.ap()[:],
                    idxs_ap=idx_sb[:, g * NIC:(g + 1) * NIC],
                    num_idxs=GRP,
                    num_idxs_reg=GRP,
                    elem_size=EPAD,
                    transpose=True,
                    queue_num=qn,
                )

            # feed order matches 1q:2f consumption (window w uses eq[w//4],
            # ef[w//2]); first-use deadlines: eq[i] at window 4i, ef[j] at 2j
            feed = sorted(
                [("q", i, 4 * i) for i in range(1, LQ // 4)] +
                [("f", j, 2 * j) for j in range(2, LH // 4)],
                key=lambda x: x[2])
            for (st, g, _) in feed:
                if st == "q":
                    gather_group(idxq_sb, g, eq, "eqg")
                else:
                    gather_group(idxf_sb, g, ef, "efg")

            # ---------- LSTM recurrence ----------
            # One step of one stream. Gate biases are folded into the x-side
            # matmul (embedding col 300 is 1.0; weight row 300 = bias; x
            # chunk 2 is an exact K=64 tile, rows 256..319).
            # Gates per hidden chunk j land in one 4-bank PSUM tile in order
            # (i, f, o, g) so the three sigmoids are one strided ACT op.
            def lstm_step(state, t, e_chunks, w_sb, label):
                h, c_st = state
                ec = e_chunks[t // 4]
                co = (t % 4) * S
                rhs_list = [ec[:, 0, co:co + S], ec[:, 1, co:co + S],
                            ec[0:KX2, 2, co:co + S]]
                nk = 3
                if t > 0:
                    rhs_list += [h[:, j, :] for j in range(4)]
                    nk = 7
                new_h = hp.tile([128, 4, S], BF16, name="hn", tag="h", bufs=4)
                new_c = cpool.tile([128, 4, S], F32, name="cn", tag="c", bufs=4)

                def mm(pg, j, ki):
                    for g in range(4):
                        m = 4 * j + g
                        nc.tensor.matmul(
                            pg[:, g, :],
                            lhsT=w_sb[0:KX2 if ki == 2 else 128, ki,
                                      m * 128:(m + 1) * 128],
                            rhs=rhs_list[ki],
                            start=(ki == 0),
                            stop=(ki == nk - 1),
                        )

                def elementwise(pg, j):
                    sig = ew.tile([128, 3, S], F32, name="sig", tag="sig", bufs=6)
                    nc.scalar.activation(
                        sig[:], pg[:, 0:3, :],
                        mybir.ActivationFunctionType.Sigmoid)
                    tg = ew.tile([128, S], F32, name="tg", tag="ew")
                    nc.scalar.activation(
                        tg[:], pg[:, 3, :], mybir.ActivationFunctionType.Tanh)
                    cn = new_c[:, j, :]
                    if t == 0:
                        nc.vector.tensor_mul(cn, sig[:, 0, :], tg[:])
                    else:
                        m1 = ew.tile([128, S], F32, name="m1", tag="ew")
                        nc.vector.tensor_mul(m1[:], sig[:, 1, :], c_st[:, j, :])
                        m2 = ew.tile([128, S], F32, name="m2", tag="ew")
                        nc.vector.tensor_mul(m2[:], sig[:, 0, :], tg[:])
                        nc.vector.tensor_add(cn, m1[:], m2[:])
                    tc_ = ew.tile([128, S], F32, name="tc", tag="ew")
                    nc.scalar.activation(
                        tc_[:], cn, mybir.ActivationFunctionType.Tanh)
                    nc.vector.tensor_mul(new_h[:, j, :], sig[:, 2, :], tc_[:])

                for j in range(4):
                    pg = ps.tile([128, 4, S], F32, name=f"pg{label}",
                                 tag="pg", padded_shape=[128, 4, 512])
                    for ki in range(nk):
                        mm(pg, j, ki)
                    elementwise(pg, j)
                return (new_h, new_c)

            # interleave 1q:2f so the two streams hide each other's
            # elementwise chains and finish together (no solo tail)
            st_q = (None, None)
            st_f = (None, None)
            for w in range(LQ):
                st_q = lstm_step(st_q, w, eq, wq_sb, "q")
                st_f = lstm_step(st_f, 2 * w, ef, wf_sb, "f")
                st_f = lstm_step(st_f, 2 * w + 1, ef, wf_sb, "f")
            hq_t = st_q[0]
            hq = [hq_t[:, j, :] for j in range(4)]
            hf_t = st_f[0]
            hf = [hf_t[:, j, :] for j in range(4)]

            # ---------- query = tanh([img, hq] @ W1.T + b1) ----------
            pq = ps.tile([128, 4, S], F32, name="pq", tag="pg",
                         padded_shape=[128, 4, 512])

            def qslice(m):
                return pq[:, m, :]

            # 16 streamed lhsT blocks of 2 k-chunks, alternating DMA queues
            # (a single in-order queue starves the matmuls at 2.9us/block)
            n_im_blk = IMG // 256
            for bI in range(n_im_blk):
                w1c = w1p.tile([128, 2, HID], BF16, name="w1c", tag="w1c",
                               bufs=4)
                eng = nc.sync if bI % 2 == 0 else nc.scalar
                eng.dma_start(
                    w1c[:],
                    w1i_d.ap()[bI * 256:(bI + 1) * 256, :].rearrange(
                        "(k p) m -> p k m", p=128))
                for k8 in range(2):
                    ki = bI * 2 + k8
                    for m in range(4):
                        nc.tensor.matmul(
                            qslice(m),
                            lhsT=w1c[:, k8, m * 128:(m + 1) * 128],
                            rhs=img_sb[:, ki, :],
                            start=(ki == 0),
                            stop=False,
                        )
            for k in range(4):
                for m in range(4):
                    nc.tensor.matmul(
                        qslice(m),
                        lhsT=w1h_sb[:, k, m * 128:(m + 1) * 128],
                        rhs=hq[k][:],
                        start=False,
                        stop=(k == 3),
                    )
            qt_f = []
            qt_b = []
            for m in range(4):
                qf = cp.tile([128, S], F32, name=f"qtf{m}", tag=f"qtf{m}")
                nc.scalar.activation(
                    qf[:], qslice(m), mybir.ActivationFunctionType.Tanh,
                    bias=b1_sb[:, m:m + 1])
                qb = cp.tile([128, S], BF16, name=f"qtb{m}", tag=f"qtb{m}")
                nc.vector.tensor_copy(qb[:], qf[:])
                qt_f.append(qf)
                qt_b.append(qb)

            # ---------- attention ----------
            # scores[n, n'] = sum_h Q[h, n] hf[h, n']  (2 partition tiles of n)
            sct = ps.tile([128, 4, S], F32, name="sct", tag="pg",
                          padded_shape=[128, 4, 512])
            sc0, sc1 = sct[:, 0, :], sct[0:S - 128, 1, :]
            for k in range(4):
                nc.tensor.matmul(sc0, lhsT=qt_b[k][:, 0:128], rhs=hf[k][:],
                                 start=(k == 0), stop=(k == 3))
            for k in range(4):
                nc.tensor.matmul(sc1, lhsT=qt_b[k][:, 128:S], rhs=hf[k][:],
                                 start=(k == 0), stop=(k == 3))

            # scores are tiny (|s| < 1: h states are ~0.03-scale), so the
            # softmax max-subtraction is skipped; exp(-1e30 mask) -> 0
            a_bf = []  # attention weights, 2 partition tiles [*, S] bf16
            for ti, (scp, npart) in enumerate([(sc0, 128), (sc1, S - 128)]):
                sm = ew.tile([128, S], F32, name="sm", tag="ew")
                nc.vector.tensor_add(sm[:npart], scp, mask_sb[:npart, ti, :])
                ex = ew.tile([128, S], F32, name="ex", tag="ew")
                nc.scalar.activation(
                    ex[:npart], sm[:npart], mybir.ActivationFunctionType.Exp)
                ssum = ew.tile([128, 1], F32, name="ssum", tag="red", bufs=4)
                nc.vector.tensor_reduce(
                    ssum[:npart], ex[:npart], mybir.AxisListType.X,
                    mybir.AluOpType.add)
                rs = ew.tile([128, 1], F32, name="rs", tag="red", bufs=4)
                nc.vector.reciprocal(rs[:npart], ssum[:npart])
                ab = ew.tile([128, S], BF16, name="ab", tag="abf", bufs=8)
                nc.vector.tensor_scalar_mul(ab[:npart], ex[:npart], rs[:npart])
                a_bf.append(ab)

            # A^T (s'-major) via PE transpose; 2 tiles covering s' 0:128, 128:160
            at = [cp.tile([128, S], BF16, name=f"at{i}", tag=f"at{i}")
                  for i in range(2)]
            blocks = [  # (src tile idx, src col slice, dst tile idx, dst col off)
                (0, 0, 128, 0, 0),
                (1, 0, 128, 0, 128),
                (0, 128, S, 1, 0),
                (1, 128, S, 1, 128),
            ]
            for (sti, c0, c1, dti, dc) in blocks:
                src = a_bf[sti]
                np_src = 128 if sti == 0 else S - 128
                w_ = c1 - c0
                pt = ps.tile([128, S], BF16, name="pt", tag="pg")
                nc.tensor.transpose(
                    pt[0:w_, 0:np_src], src[0:np_src, c0:c1],
                    ident[0:np_src, 0:np_src])
                nc.vector.tensor_copy(
                    at[dti][0:w_, dc:dc + np_src], pt[0:w_, 0:np_src])

            # hf token-major [S, 512] as 2 partition tiles
            hft = [cp.tile([128, 4, 128], BF16, name=f"hft{i}", tag=f"hft{i}")
                   for i in range(2)]
            for k in range(4):
                pt = ps.tile([128, S], BF16, name="pt2", tag="pg")
                nc.tensor.transpose(
                    pt[0:128, 0:128], hf[k][:, 0:128], ident[:])
                nc.vector.tensor_copy(hft[0][:, k, :], pt[0:128, 0:128])
                pt = ps.tile([128, S], BF16, name="pt3", tag="pg")
                nc.tensor.transpose(
                    pt[0:S - 128, 0:128], hf[k][:, 128:S], ident[:])
                nc.vector.tensor_copy(
                    hft[1][0:S - 128, k, :], pt[0:S - 128, 0:128])

            # att_hist^T [512, S] = hf^T(feature-major result) : contract over s'
            att_b = []
            pa = ps.tile([128, 4, S], F32, name="pa", tag="pg",
                         padded_shape=[128, 4, 512])
            for m in range(4):
                nc.tensor.matmul(pa[:, m, :], lhsT=hft[0][:, m, :], rhs=at[0][:],
                                 start=True, stop=False)
                nc.tensor.matmul(pa[:, m, :], lhsT=hft[1][0:S - 128, m, :],
                                 rhs=at[1][0:S - 128, :],
                                 start=False, stop=True)
                ab2 = ew.tile([128, S], BF16, name="ab2", tag="abf", bufs=8)
                nc.vector.tensor_copy(ab2[:], pa[:, m, :])
                att_b.append(ab2)

            # out = Q + tanh(att @ W2.T + b2), feature-major [512, S]
            po = ps.tile([128, 4, S], F32, name="po", tag="pg",
                         padded_shape=[128, 4, 512])
            for m in range(4):
                for k in range(4):
                    nc.tensor.matmul(
                        po[:, m, :],
                        lhsT=w2_sb[:, k, m * 128:(m + 1) * 128],
                        rhs=att_b[k][:],
                        start=(k == 0), stop=(k == 3))
                th = ew.tile([128, S], F32, name="th", tag="ew")
                nc.scalar.activation(
                    th[:], po[:, m, :], mybir.ActivationFunctionType.Tanh,
                    bias=b2_sb[:, m:m + 1])
                om = op.tile([128, S], F32, name="om", tag="om")
                nc.vector.tensor_add(om[:], th[:], qt_f[m][:])
                nc.sync.dma_start(out_d.ap()[m * 128:(m + 1) * 128, :], om[:])

    nc.compile()
    return nc


def _prep_shared(inp):
    f32 = np.float32
    emb = np.asarray(inp["emb"], f32)
    embp = np.zeros((VOCAB, EPAD), NP_BF16)
    embp[:, :EMB] = emb.astype(NP_BF16)
    embp[0, :] = 0
    embp[:, EMB] = 1  # ones column (feature 300): x-matmul adds the bias row

    perm = _gate_perm()

    def fuse_w(wih, whh, bih, bhh):
        wx = np.zeros((EPAD, G4), f32)
        wx[0:EMB, :] = np.asarray(wih, f32).T
        wx[EMB, :] = np.asarray(bih, f32) + np.asarray(bhh, f32)
        wh = np.ascontiguousarray(np.asarray(whh, f32).T)
        return (np.ascontiguousarray(wx[:, perm]).astype(NP_BF16),
                np.ascontiguousarray(wh[:, perm]).astype(NP_BF16))

    wqx, wqh = fuse_w(inp["Wih_q"], inp["Whh_q"], inp["bih_q"], inp["bhh_q"])
    wfx, wfh = fuse_w(inp["Wih_f"], inp["Whh_f"], inp["bih_f"], inp["bhh_f"])
    W1 = np.asarray(inp["W1"], f32)
    shared = {
        "wqx": wqx, "wqh": wqh, "wfx": wfx, "wfh": wfh,
        "w1i": np.ascontiguousarray(W1[:, :IMG].T).astype(NP_BF16),
        "w1h": np.ascontiguousarray(W1[:, IMG:].T).astype(NP_BF16),
        "b1": np.ascontiguousarray(
            np.asarray(inp["b1"], f32).reshape(4, 128).T),
        "w2": np.ascontiguousarray(np.asarray(inp["W2"], f32).T).astype(NP_BF16),
        "b2": np.ascontiguousarray(
            np.asarray(inp["b2"], f32).reshape(4, 128).T),
        "_embp": embp,
    }
    n = np.arange(S)
    mask = np.where(
        (n[:, None] // R == n[None, :] // R) & (n[None, :] % R <= n[:, None] % R),
        np.float32(0.0), np.float32(NEG))
    shared["mask"] = np.ascontiguousarray(mask.astype(f32))
    return shared


def _prep_core(inp, core, embp):
    sl = slice(core * BS, (core + 1) * BS)

    def flat(arr, L):
        # t-major flat order i = t*S + n; dma_gather reads index i from
        # [i % 16, base + i // 16], 16-partition block replicated to 128
        return np.asarray(arr[sl], np.int64).reshape(S, L).T.reshape(-1)

    qf = flat(inp["questions"], LQ)          # [3200]
    ff = flat(inp["history"], LH)            # [6400]
    uniq, inv = np.unique(np.concatenate([qf, ff]), return_inverse=True)
    assert len(uniq) <= NU_PAD
    embC = np.zeros((NU_PAD, EPAD), NP_BF16)
    embC[:len(uniq)] = embp[uniq]
    inv = inv.astype(np.int16)

    def wrap(x):
        w = x.reshape(-1, 16).T                       # [16, L*S/16]
        return np.ascontiguousarray(np.tile(w, (8, 1)))

    def pregather(flat_idx):
        # same layout dma_gather(transpose=True) writes: [128, 3, GRP] with
        # out[p, c, i] = row_i[c*128 + p]
        rows = embC[flat_idx]                      # [GRP, 384]
        return np.ascontiguousarray(
            rows.reshape(-1, 3, 128).transpose(2, 1, 0).reshape(128, -1))

    GRP = 4 * S
    qi = inv[:LQ * S]
    fi = inv[LQ * S:]
    img = np.asarray(inp["img_features"], np.float32)[sl]          # [16, 4096]
    img_rep = np.repeat(img, R, axis=0).T                          # [4096, 160]
    return {
        "embC": embC,
        "idxq": wrap(qi),
        "idxf": wrap(fi),
        "e0q": pregather(qi[:GRP]),
        "e0f": pregather(fi[:GRP]),
        "e1f": pregather(fi[GRP:2 * GRP]),
        "imgrep": np.ascontiguousarray(img_rep).astype(NP_BF16),
    }


def kernel(**inputs) -> np.ndarray:
    global _STATE
    if _STATE is None:
        _STATE = _build_program()
    nc = _STATE

    shared = _prep_shared(inputs)
    embp = shared.pop("_embp")
    in_maps = []
    for c in range(N_CORES):
        m = dict(shared)
        m.update(_prep_core(inputs, c, embp))
        in_maps.append(m)

    res = run_bass_kernel_spmd(nc, in_maps, core_ids=list(range(N_CORES)))
    outs = []
    for c in range(N_CORES):
        o = np.asarray(res.results[c]["out"], np.float32)   # [512, 160]
        outs.append(o.T.reshape(BS, R, HID))
    return np.concatenate(outs, axis=0)                      # [128, 10, 512]
